# revision 2
# baseline (speedup 1.0000x reference)
"""Multi-head attention (B=2, S=2048, D=1024, H=16) on 8 Trainium2 cores.

Sharding: core c handles batch c//4 and head-group c%4 (4 heads x dk 64).
Pipeline:
  1. x loaded in 512-token quarter DMAs (one DMA per tensor-quarter),
     weights in one DMA per tensor.  Only the projections the first score
     tiles need run up front; K quarters 1-3, V, and Q quarters 1-3 are
     interleaved into early attention sub-blocks.
  2. Q/K projections -> [128(2 heads), 2048] feature-major layout, bias
     fused into the PSUM->SBUF DVE copy.  V projection -> token-major
     [128 tok, 4x(64+1)] with a ones column per head (softmax denominator).
  3. Attention in 16 sub-blocks (512-query chunk x head), software
     pipelined with variable depth: the exp stream runs several sub-blocks
     ahead early (while the PE drains projection work), consumers catch up
     to depth 1 mid-stream.  Scores for a key-tile pair share one
     [128, 1024] PSUM pair -> one exp per pair.
  4. Wo partial projection (256 local features -> full D) + bo/4 bias per
     512-token chunk, staged to DRAM, one ReduceScatter per chunk across
     the 4-core batch group (4 staggered collectives; only the last is
     exposed).  RS writes bf16 output params directly; host upcasts.
"""

import numpy as np
import ml_dtypes

import concourse.bass as bass
import concourse.tile as tile
from concourse import bacc, mybir
from concourse.bass_utils import run_bass_kernel_spmd

BF16 = mybir.dt.bfloat16
F32 = mybir.dt.float32
NPBF16 = ml_dtypes.bfloat16

B, S, D, H = 2, 2048, 1024, 16
DK = 64
N_CORES = 8
HPC = 4                 # heads per core
FEAT = HPC * DK         # 256 projected features per core
VW = DK + 1             # 65: per-head v columns incl ones
TOKQ = 512              # token quarter for x DMAs / projections
QC = 512                # query chunk per attention sub-block
NCH = S // QC           # 4 query chunks
NKC = D // 128          # 8 contraction chunks
NKT = S // 128          # 16 key tiles
RSTOK = QC // 4         # 128 tokens per core per ReduceScatter

_CACHE = {}


def _build_program():
    if "nc" in _CACHE:
        return _CACHE["nc"]

    from concourse.masks import make_identity

    nc = bacc.Bacc("TRN2", target_bir_lowering=False, debug=False,
                   num_devices=N_CORES)

    xq = nc.declare_dram_parameter("xq", [D, S], BF16, isOutput=False)
    xk = nc.declare_dram_parameter("xk", [D, S], BF16, isOutput=False)
    xv = nc.declare_dram_parameter("xv", [D, S], BF16, isOutput=False)
    wq = nc.declare_dram_parameter("wq", [D, FEAT], BF16, isOutput=False)
    wk = nc.declare_dram_parameter("wk", [D, FEAT], BF16, isOutput=False)
    wv = nc.declare_dram_parameter("wv", [D, HPC * VW], BF16, isOutput=False)
    wo = nc.declare_dram_parameter("wo", [FEAT, D], BF16, isOutput=False)
    # merged per-partition bias columns: bq(2) | bk(2) | bo/4(8)
    bqko = nc.declare_dram_parameter("bqko", [128, 12], F32, isOutput=False)
    bv = nc.declare_dram_parameter("bv", [1, HPC * VW], BF16, isOutput=False)
    # ReduceScatter writes each 128-token output piece directly; host
    # upcasts bf16 -> f32
    outs = [nc.declare_dram_parameter(f"out{r}", [D, RSTOK], BF16,
                                      isOutput=True) for r in range(NCH)]

    with tile.TileContext(nc) as tc:
        with (
            tc.tile_pool(name="w", bufs=1) as wpool,
            tc.tile_pool(name="x", bufs=2) as xpool,
            tc.tile_pool(name="qk", bufs=1) as qkpool,
            tc.tile_pool(name="sc", bufs=42) as scpool,
            tc.tile_pool(name="sm", bufs=4) as smpool,
            tc.tile_pool(name="cat", bufs=1) as catpool,
            tc.tile_pool(name="fb", bufs=3) as fbpool,
            tc.tile_pool(name="ps_sc", bufs=2, space="PSUM") as ps_sc,
            tc.tile_pool(name="ps_pv", bufs=2, space="PSUM") as ps_pv,
            tc.tile_pool(name="ps_mm", bufs=2, space="PSUM") as ps_mm,
            tc.tile_pool(name="dram", bufs=1, space="DRAM") as dram,
        ):
            # ---- constants / weights (one DMA per tensor) ----------------
            ident = wpool.tile([128, 128], BF16, tag="ident")
            make_identity(nc, ident[:])
            ones1 = wpool.tile([1, 128], BF16, tag="ones")
            nc.vector.memset(ones1[:], 1.0)

            def kc_load(name, dramt, width):
                t = wpool.tile([128, NKC * width], BF16, tag=name, name=name)
                nc.sync.dma_start(
                    t[:].rearrange("p (kc f) -> p kc f", f=width),
                    dramt[:].rearrange("(kc p) f -> p kc f", p=128))
                return t

            x_tiles = {}  # (tensor_idx, quarter) -> tile

            def x_dma(ti, xdram, qtr):
                src = xdram[:].rearrange("(kc p) t -> p kc t", p=128)
                t = xpool.tile([128, NKC * TOKQ], BF16, tag=f"x{ti}",
                               name=f"x{ti}_{qtr}")
                nc.sync.dma_start(
                    t[:].rearrange("p (kc t) -> p kc t", t=TOKQ),
                    src[:, :, bass.ts(qtr, TOKQ)])
                x_tiles[(ti, qtr)] = t

            # DMA stream ordered so the first score matmuls start early;
            # later K/V/Q quarters arrive while attention runs.
            wk_sb = kc_load("wks", wk, FEAT)
            x_dma(1, xk, 0)
            wq_sb = kc_load("wqs", wq, FEAT)
            bqko_sb = wpool.tile([128, 12], F32, tag="bqko")
            nc.sync.dma_start(bqko_sb[:], bqko[:])
            bq_sb = bqko_sb[:, 0:2]
            bk_sb = bqko_sb[:, 2:4]
            bo4_sb = bqko_sb[:, 4:12]
            x_dma(0, xq, 0)
            for qtr in range(1, 4):
                x_dma(1, xk, qtr)
            x_dma(0, xq, 1)
            wv_sb = kc_load("wvs", wv, HPC * VW)
            bv_sb = wpool.tile([1, HPC * VW], BF16, tag="bv")
            nc.sync.dma_start(bv_sb[:], bv[:])
            for qtr in range(4):
                x_dma(2, xv, qtr)
            wo_sb = wpool.tile([128, 2 * D], BF16, tag="wos")
            nc.sync.dma_start(
                wo_sb[:].rearrange("p (kc f) -> p kc f", f=D),
                wo[:].rearrange("(kc p) f -> p kc f", p=128))
            for qtr in range(2, 4):
                x_dma(0, xq, qtr)

            # ---- projections --------------------------------------------
            qh_sb = [qkpool.tile([128, S], BF16, tag=f"qh{m}", name=f"qh{m}")
                     for m in range(2)]
            kh_sb = [qkpool.tile([128, S], BF16, tag=f"kh{m}", name=f"kh{m}")
                     for m in range(2)]
            v_big = qkpool.tile([128, NKT * HPC * VW], BF16, tag="vb")

            def qk_proj(w_sb, ti, b_sb, dst, qtr):
                xt = x_tiles[(ti, qtr)]
                for m in range(2):
                    ps = ps_mm.tile([128, TOKQ], F32, tag="mm", name="qkps")
                    for kc in range(NKC):
                        nc.tensor.matmul(
                            ps[:],
                            w_sb[:, kc * FEAT + m * 128:kc * FEAT + (m + 1) * 128],
                            xt[:, bass.ts(kc, TOKQ)],
                            start=(kc == 0), stop=(kc == NKC - 1),
                        )
                    nc.vector.tensor_scalar_add(
                        dst[m][:, bass.ts(qtr, TOKQ)], ps[:], b_sb[:, m:m + 1])

            def v_proj_j(qtr, j):
                """One 128-token group of the V projection; bias added via
                partition-broadcast in the PSUM->SBUF copy."""
                xt = x_tiles[(2, qtr)]
                ps = ps_mm.tile([128, TOKQ], F32, tag="mm", name="vps")
                for kc in range(NKC):
                    nc.tensor.matmul(
                        ps[:, 0:HPC * VW],
                        xt[:, kc * TOKQ + j * 128:kc * TOKQ + (j + 1) * 128],
                        wv_sb[:, kc * HPC * VW:(kc + 1) * HPC * VW],
                        start=(kc == 0), stop=False,
                    )
                nc.tensor.matmul(ps[:, 0:HPC * VW], ones1[:], bv_sb[:],
                                 start=False, stop=True)
                kt = qtr * 4 + j
                nc.vector.tensor_copy(
                    v_big[:, kt * HPC * VW:(kt + 1) * HPC * VW],
                    ps[:, 0:HPC * VW])

            # up-front: only what sub-block 0 needs
            qk_proj(wk_sb, 1, bk_sb, kh_sb, 0)
            qk_proj(wq_sb, 0, bq_sb, qh_sb, 0)

            # ---- attention + output projection, software-pipelined ------
            rs_in = [dram.tile([4 * D, RSTOK], BF16, tag=f"rsi{r}",
                               name=f"rsi{r}") for r in range(NCH)]
            rs_out = [dram.tile([D, RSTOK], BF16, tag=f"rso{r}",
                                name=f"rso{r}") for r in range(NCH)]

            concat = {}
            for ch in range(NCH):
                for kc in range(2):
                    concat[(ch, kc)] = catpool.tile(
                        [128, QC], BF16, tag=f"cat{ch}{kc}",
                        name=f"cat{ch}{kc}")

            def emit_scores(ch, h, ktp):
                """Scores for key tiles (2*ktp, 2*ktp+1) over this 512-query
                chunk; one exp for the pair."""
                hp, hr = h // 2, (h % 2) * 64
                ps = ps_sc.tile([128, 2 * QC], F32, tag="sc", name="scps")
                for half in range(2):
                    kt = 2 * ktp + half
                    nc.tensor.matmul(
                        ps[:, bass.ts(half, QC)],
                        kh_sb[hp][hr:hr + 64, bass.ts(kt, 128)],
                        qh_sb[hp][hr:hr + 64, bass.ts(ch, QC)],
                        start=True, stop=True,
                    )
                sc = scpool.tile([128, 2 * QC], BF16, tag="sc", name="sc")
                nc.scalar.activation(
                    sc[:], ps[:], mybir.ActivationFunctionType.Exp,
                    scale=0.125)
                return sc

            class Sub:
                def __init__(self, ch, h, sc2):
                    self.ch, self.h, self.sc2 = ch, h, sc2
                    self.pv = None

            def emit_consumer_step(st, j):
                """PV accumulation for 128-query block j of a finished
                sub-block, then normalize + transpose + concat store."""
                ch, h = st.ch, st.h
                hp, hr = h // 2, (h % 2) * 64
                if st.pv is None:
                    st.pv = ps_pv.tile([128, QC], F32, tag="pv", name="pv")
                pv = st.pv
                for kt in range(NKT):
                    nc.tensor.matmul(
                        pv[:, j * 128:j * 128 + VW],
                        st.sc2[kt // 2][:, (kt % 2) * QC + j * 128:
                                        (kt % 2) * QC + (j + 1) * 128],
                        v_big[:, kt * HPC * VW + h * VW:
                              kt * HPC * VW + h * VW + VW],
                        start=(kt == 0), stop=(kt == NKT - 1),
                    )
                r = smpool.tile([128, 1], F32, tag="r8", name="r8")
                nc.vector.reciprocal(r[:], pv[:, j * 128 + DK:j * 128 + DK + 1])
                onrm = smpool.tile([128, DK], BF16, tag="onrm", name="onrm")
                nc.vector.tensor_scalar_mul(
                    onrm[:], pv[:, j * 128:j * 128 + DK], r[:])
                tp = ps_mm.tile([128, 128], BF16, tag="mm", name="tp")
                nc.tensor.transpose(tp[hr:hr + 64, :], onrm[:], ident[:])
                nc.vector.tensor_copy(
                    concat[(ch, hp)][hr:hr + 64, bass.ts(j, 128)],
                    tp[hr:hr + 64, :])

            def emit_wo_b(ch):
                """Second half (heads 2-3) + combine + staging DMAs + RS."""
                dst = rs_in[ch][:].rearrange(
                    "(s mp p) t -> p mp s t", s=4, p=128)
                for m in range(NKC):
                    fo = fbpool.tile([128, QC], BF16, tag="fob", name="fob")
                    ps = ps_mm.tile([128, QC], F32, tag="mm", name="wobps")
                    for kc in range(2):
                        nc.tensor.matmul(
                            ps[:], wo_sb[:, kc * D + m * 128:kc * D + (m + 1) * 128],
                            concat[(ch, kc)][:],
                            start=(kc == 0), stop=(kc == 1),
                        )
                    if ch == NCH - 1 and m % 2:
                        nc.scalar.activation(
                            fo[:], ps[:],
                            mybir.ActivationFunctionType.Identity,
                            bias=bo4_sb[:, m:m + 1])
                    else:
                        nc.vector.tensor_scalar_add(fo[:], ps[:],
                                                    bo4_sb[:, m:m + 1])
                    nc.sync.dma_start(
                        dst[:, m, :, :],
                        fo[:].rearrange("p (s t) -> p s t", s=4))
                nc.gpsimd.collective_compute(
                    "ReduceScatter", mybir.AluOpType.add,
                    replica_groups=[[0, 1, 2, 3], [4, 5, 6, 7]],
                    ins=[rs_in[ch][:].opt()],
                    outs=[rs_out[ch][:].opt()],
                )
                # collectives may not write IO tensors; bounce via DRAM on
                # the gpsimd queue (its head is already gated on this RS)
                nc.gpsimd.dma_start(outs[ch][:], rs_out[ch][:])

            # extras[(sub, slot)] -> emissions filling early PE gaps while
            # respecting DMA arrival order
            extras = {
                (0, 0): [lambda: qk_proj(wk_sb, 1, bk_sb, kh_sb, 1)],
                (0, 2): [lambda: qk_proj(wk_sb, 1, bk_sb, kh_sb, 2)],
                (0, 4): [lambda: qk_proj(wk_sb, 1, bk_sb, kh_sb, 3)],
                (1, 0): [lambda: qk_proj(wq_sb, 0, bq_sb, qh_sb, 1)],
                (6, 0): [lambda: qk_proj(wq_sb, 0, bq_sb, qh_sb, 2)],
                (10, 0): [lambda: qk_proj(wq_sb, 0, bq_sb, qh_sb, 3)],
            }
            for i in range(16):   # V-proj 128-token groups over subs 2-3
                extras.setdefault((2 + i // 8, i % 8), []).append(
                    (lambda q, jj: lambda: v_proj_j(q, jj))(i // 4, i % 4))

            # sub-block order: chunk-major, heads inner.  Consumers: none
            # for subs 0-3 (projection catch-up), two per sub for 4-6, one
            # per sub afterwards.
            subs = [(ch, h) for ch in range(NCH) for h in range(HPC)]
            pending = []      # emitted-scores sub-blocks awaiting consumers
            done_h = {ch: 0 for ch in range(NCH)}

            def consume_one():
                st = pending.pop(0)
                for j in range(QC // 128):
                    emit_consumer_step(st, j)
                done_h[st.ch] += 1
                if done_h[st.ch] == HPC:
                    emit_wo_b(st.ch)

            # consume schedule: none during projection catch-up (subs 0-3),
            # one per sub mid-stream (depth ~4), two per sub near the end
            # (subs 12-14) so only sub 15's consumer trails the exp stream
            for si, (ch, h) in enumerate(subs):
                sc2 = []
                n_consume = 0 if si < 4 else (2 if 12 <= si < 15 else 1)
                for ktp in range(NKT // 2):
                    sc2.append(emit_scores(ch, h, ktp))
                    for fn in extras.get((si, ktp), ()):
                        fn()
                    if n_consume and ktp in ((1, 5) if n_consume == 2 else (5,)):
                        consume_one()
                pending.append(Sub(ch, h, sc2))
            while pending:
                consume_one()

    nc.compile()
    _CACHE["nc"] = nc
    return nc


def _prep_inputs(q, k, v, Wq, bq, Wk, bk, Wv, bv, Wo, bo):
    """Build the per-core input maps (host-side sharding)."""
    in_maps = []
    for c in range(N_CORES):
        b, hg = c // 4, c % 4
        fsl = slice(FEAT * hg, FEAT * (hg + 1))
        wv_aug = np.zeros((D, HPC * VW), np.float32)
        bv_aug = np.zeros((HPC * VW,), np.float32)
        for h in range(HPC):
            rows = slice(FEAT * hg + DK * h, FEAT * hg + DK * (h + 1))
            wv_aug[:, h * VW:h * VW + DK] = Wv[rows, :].T
            bv_aug[h * VW:h * VW + DK] = bv[rows]
            bv_aug[h * VW + DK] = 1.0
        bqko = np.concatenate([
            bq[fsl].reshape(2, 128).T,
            bk[fsl].reshape(2, 128).T,
            (bo * 0.25).reshape(8, 128).T,
        ], axis=1)
        in_maps.append({
            "xq": np.ascontiguousarray(q[b].T).astype(NPBF16),
            "xk": np.ascontiguousarray(k[b].T).astype(NPBF16),
            "xv": np.ascontiguousarray(v[b].T).astype(NPBF16),
            "wq": np.ascontiguousarray(Wq[fsl].T).astype(NPBF16),
            "wk": np.ascontiguousarray(Wk[fsl].T).astype(NPBF16),
            "wv": wv_aug.astype(NPBF16),
            "wo": np.ascontiguousarray(Wo[:, fsl].T).astype(NPBF16),
            "bqko": np.ascontiguousarray(bqko).astype(np.float32),
            "bv": bv_aug.reshape(1, HPC * VW).astype(NPBF16),
        })
    return in_maps


def run_sharded(in_maps, trace=False):
    nc = _build_program()
    res = run_bass_kernel_spmd(nc, in_maps, list(range(N_CORES)), trace=trace)
    full = np.empty((B, S, D), np.float32)
    for c in range(N_CORES):
        b, hg = c // 4, c % 4
        for ch in range(NCH):
            t0 = QC * ch + RSTOK * hg
            full[b, t0:t0 + RSTOK, :] = \
                res.results[c][f"out{ch}"].astype(np.float32).T
    return full, res


def kernel(q, k, v, Wq, bq, Wk, bk, Wv, bv, Wo, bo):
    args = [np.asarray(x, np.float32) for x in
            (q, k, v, Wq, bq, Wk, bk, Wv, bv, Wo, bo)]
    in_maps = _prep_inputs(*args)
    full, _ = run_sharded(in_maps)
    return full


# revision 3
# speedup vs baseline: 1.0278x; 1.0278x over previous
"""Multi-head attention (B=2, S=2048, D=1024, H=16) on 8 Trainium2 cores, v3.

Sharding: core c handles batch c//4 and head-group c%4 (4 heads x dk 64).
Pipeline:
  1. x loaded in 512-token quarter DMAs (one DMA per tensor-quarter),
     weights in one DMA per tensor.  Only the projections the first score
     tiles need run up front; K quarters 1-3, V, and Q quarters 1-3 are
     interleaved into early attention sub-blocks.
  2. Q/K projections -> [128(2 heads), 2048] feature-major layout, bias
     fused into the PSUM->SBUF DVE copy.  V projection -> token-major
     [128 tok, 4x(64+1)] with a ones column per head (softmax denominator).
  3. Attention in 16 sub-blocks (512-query chunk x head), software
     pipelined with variable depth: the exp stream runs several sub-blocks
     ahead early (while the PE drains projection work), consumers catch up
     to depth 1 mid-stream.  Scores for a key-tile pair share one
     [128, 1024] PSUM pair -> one exp per pair.
  4. Wo partial projection (256 local features -> full D) + bo/4 bias per
     512-token chunk, staged to DRAM, one ReduceScatter per chunk across
     the 4-core batch group (4 staggered collectives; only the last is
     exposed).  RS writes bf16 output params directly; host upcasts.
"""

import numpy as np
import ml_dtypes

import concourse.bass as bass
import concourse.tile as tile
from concourse import bacc, mybir
from concourse.bass_utils import run_bass_kernel_spmd

BF16 = mybir.dt.bfloat16
F32 = mybir.dt.float32
NPBF16 = ml_dtypes.bfloat16

B, S, D, H = 2, 2048, 1024, 16
DK = 64
N_CORES = 8
HPC = 4                 # heads per core
FEAT = HPC * DK         # 256 projected features per core
VW = DK + 1             # 65: per-head v columns incl ones
TOKQ = 512              # token quarter for x DMAs / projections
QC = 512                # query chunk per attention sub-block
NCH = S // QC           # 4 query chunks
NKC = D // 128          # 8 contraction chunks
NKT = S // 128          # 16 key tiles
RSTOK = QC // 4         # 128 tokens per core per ReduceScatter

_CACHE = {}


def _build_program():
    if "nc" in _CACHE:
        return _CACHE["nc"]

    from concourse.masks import make_identity

    nc = bacc.Bacc("TRN2", target_bir_lowering=False, debug=False,
                   num_devices=N_CORES)

    xq = nc.declare_dram_parameter("xq", [D, S], BF16, isOutput=False)
    xk = nc.declare_dram_parameter("xk", [D, S], BF16, isOutput=False)
    xv = nc.declare_dram_parameter("xv", [D, S], BF16, isOutput=False)
    wq = nc.declare_dram_parameter("wq", [D, FEAT], BF16, isOutput=False)
    wk = nc.declare_dram_parameter("wk", [D, FEAT], BF16, isOutput=False)
    wv = nc.declare_dram_parameter("wv", [D, HPC * VW], BF16, isOutput=False)
    wo = nc.declare_dram_parameter("wo", [FEAT, D], BF16, isOutput=False)
    # merged per-partition bias columns: bq(2) | bk(2) | bo/4(8)
    bqko = nc.declare_dram_parameter("bqko", [128, 12], F32, isOutput=False)
    bv = nc.declare_dram_parameter("bv", [1, HPC * VW], BF16, isOutput=False)
    # ReduceScatter writes each 128-token output piece directly; host
    # upcasts bf16 -> f32
    outs = [nc.declare_dram_parameter(f"out{r}", [D, RSTOK], BF16,
                                      isOutput=True) for r in range(NCH)]

    with tile.TileContext(nc) as tc:
        with (
            tc.tile_pool(name="w", bufs=1) as wpool,
            tc.tile_pool(name="x", bufs=2) as xpool,
            tc.tile_pool(name="qk", bufs=1) as qkpool,
            tc.tile_pool(name="sc", bufs=42) as scpool,
            tc.tile_pool(name="sm", bufs=4) as smpool,
            tc.tile_pool(name="cat", bufs=1) as catpool,
            tc.tile_pool(name="fb", bufs=9) as fbpool,
            tc.tile_pool(name="ps_sc", bufs=2, space="PSUM") as ps_sc,
            tc.tile_pool(name="ps_pv", bufs=2, space="PSUM") as ps_pv,
            tc.tile_pool(name="ps_mm", bufs=2, space="PSUM") as ps_mm,
            tc.tile_pool(name="dram", bufs=1, space="DRAM") as dram,
        ):
            # ---- constants / weights (one DMA per tensor) ----------------
            ident = wpool.tile([128, 128], BF16, tag="ident")
            make_identity(nc, ident[:])
            ones1 = wpool.tile([1, 128], BF16, tag="ones")
            nc.vector.memset(ones1[:], 1.0)

            def kc_load(name, dramt, width):
                t = wpool.tile([128, NKC * width], BF16, tag=name, name=name)
                nc.sync.dma_start(
                    t[:].rearrange("p (kc f) -> p kc f", f=width),
                    dramt[:].rearrange("(kc p) f -> p kc f", p=128))
                return t

            x_tiles = {}  # (tensor_idx, quarter) -> tile

            def x_dma(ti, xdram, qtr):
                src = xdram[:].rearrange("(kc p) t -> p kc t", p=128)
                t = xpool.tile([128, NKC * TOKQ], BF16, tag=f"x{ti}",
                               name=f"x{ti}_{qtr}")
                nc.sync.dma_start(
                    t[:].rearrange("p (kc t) -> p kc t", t=TOKQ),
                    src[:, :, bass.ts(qtr, TOKQ)])
                x_tiles[(ti, qtr)] = t

            # DMA stream ordered so the first score matmuls start early;
            # later K/V/Q quarters arrive while attention runs.
            wk_sb = kc_load("wks", wk, FEAT)
            x_dma(1, xk, 0)
            wq_sb = kc_load("wqs", wq, FEAT)
            bqko_sb = wpool.tile([128, 12], F32, tag="bqko")
            nc.sync.dma_start(bqko_sb[:], bqko[:])
            bq_sb = bqko_sb[:, 0:2]
            bk_sb = bqko_sb[:, 2:4]
            bo4_sb = bqko_sb[:, 4:12]
            x_dma(0, xq, 0)
            for qtr in range(1, 4):
                x_dma(1, xk, qtr)
            x_dma(0, xq, 1)
            wv_sb = kc_load("wvs", wv, HPC * VW)
            bv_sb = wpool.tile([1, HPC * VW], BF16, tag="bv")
            nc.sync.dma_start(bv_sb[:], bv[:])
            for qtr in range(4):
                x_dma(2, xv, qtr)
            wo_sb = wpool.tile([128, 2 * D], BF16, tag="wos")
            nc.sync.dma_start(
                wo_sb[:].rearrange("p (kc f) -> p kc f", f=D),
                wo[:].rearrange("(kc p) f -> p kc f", p=128))
            for qtr in range(2, 4):
                x_dma(0, xq, qtr)

            # ---- projections --------------------------------------------
            qh_sb = [qkpool.tile([128, S], BF16, tag=f"qh{m}", name=f"qh{m}")
                     for m in range(2)]
            kh_sb = [qkpool.tile([128, S], BF16, tag=f"kh{m}", name=f"kh{m}")
                     for m in range(2)]
            v_big = qkpool.tile([128, NKT * HPC * VW], BF16, tag="vb")

            def qk_proj(w_sb, ti, b_sb, dst, qtr):
                xt = x_tiles[(ti, qtr)]
                for m in range(2):
                    ps = ps_mm.tile([128, TOKQ], F32, tag="mm", name="qkps")
                    for kc in range(NKC):
                        nc.tensor.matmul(
                            ps[:],
                            w_sb[:, kc * FEAT + m * 128:kc * FEAT + (m + 1) * 128],
                            xt[:, bass.ts(kc, TOKQ)],
                            start=(kc == 0), stop=(kc == NKC - 1),
                        )
                    nc.vector.tensor_scalar_add(
                        dst[m][:, bass.ts(qtr, TOKQ)], ps[:], b_sb[:, m:m + 1])

            def v_proj_j(qtr, j):
                """One 128-token group of the V projection; bias added via
                partition-broadcast in the PSUM->SBUF copy."""
                xt = x_tiles[(2, qtr)]
                ps = ps_mm.tile([128, TOKQ], F32, tag="mm", name="vps")
                for kc in range(NKC):
                    nc.tensor.matmul(
                        ps[:, 0:HPC * VW],
                        xt[:, kc * TOKQ + j * 128:kc * TOKQ + (j + 1) * 128],
                        wv_sb[:, kc * HPC * VW:(kc + 1) * HPC * VW],
                        start=(kc == 0), stop=False,
                    )
                nc.tensor.matmul(ps[:, 0:HPC * VW], ones1[:], bv_sb[:],
                                 start=False, stop=True)
                kt = qtr * 4 + j
                nc.vector.tensor_copy(
                    v_big[:, kt * HPC * VW:(kt + 1) * HPC * VW],
                    ps[:, 0:HPC * VW])

            # up-front: only what sub-block 0 needs
            qk_proj(wk_sb, 1, bk_sb, kh_sb, 0)
            qk_proj(wq_sb, 0, bq_sb, qh_sb, 0)

            # ---- attention + output projection, software-pipelined ------
            rs_in = [dram.tile([4 * D, RSTOK], BF16, tag=f"rsi{r}",
                               name=f"rsi{r}") for r in range(NCH)]
            rs_out = [dram.tile([D, RSTOK], BF16, tag=f"rso{r}",
                                name=f"rso{r}") for r in range(NCH)]

            concat = {}
            for ch in range(NCH):
                for kc in range(2):
                    concat[(ch, kc)] = catpool.tile(
                        [128, QC], BF16, tag=f"cat{ch}{kc}",
                        name=f"cat{ch}{kc}")

            def emit_scores(ch, h, ktp):
                """Scores for key tiles (2*ktp, 2*ktp+1) over this 512-query
                chunk; one exp for the pair."""
                hp, hr = h // 2, (h % 2) * 64
                ps = ps_sc.tile([128, 2 * QC], F32, tag="sc", name="scps")
                for half in range(2):
                    kt = 2 * ktp + half
                    nc.tensor.matmul(
                        ps[:, bass.ts(half, QC)],
                        kh_sb[hp][hr:hr + 64, bass.ts(kt, 128)],
                        qh_sb[hp][hr:hr + 64, bass.ts(ch, QC)],
                        start=True, stop=True,
                    )
                sc = scpool.tile([128, 2 * QC], BF16, tag="sc", name="sc")
                nc.scalar.activation(
                    sc[:], ps[:], mybir.ActivationFunctionType.Exp,
                    scale=0.125)
                return sc

            class Sub:
                def __init__(self, ch, h, sc2):
                    self.ch, self.h, self.sc2 = ch, h, sc2
                    self.pv = None

            def emit_consumer_step(st, j):
                """PV accumulation for 128-query block j of a finished
                sub-block, then normalize + transpose + concat store."""
                ch, h = st.ch, st.h
                hp, hr = h // 2, (h % 2) * 64
                if st.pv is None:
                    st.pv = ps_pv.tile([128, QC], F32, tag="pv", name="pv")
                pv = st.pv
                for kt in range(NKT):
                    nc.tensor.matmul(
                        pv[:, j * 128:j * 128 + VW],
                        st.sc2[kt // 2][:, (kt % 2) * QC + j * 128:
                                        (kt % 2) * QC + (j + 1) * 128],
                        v_big[:, kt * HPC * VW + h * VW:
                              kt * HPC * VW + h * VW + VW],
                        start=(kt == 0), stop=(kt == NKT - 1),
                    )
                r = smpool.tile([128, 1], F32, tag="r8", name="r8")
                nc.vector.reciprocal(r[:], pv[:, j * 128 + DK:j * 128 + DK + 1])
                onrm = smpool.tile([128, DK], BF16, tag="onrm", name="onrm")
                nc.vector.tensor_scalar_mul(
                    onrm[:], pv[:, j * 128:j * 128 + DK], r[:])
                tp = ps_mm.tile([128, 128], BF16, tag="mm", name="tp")
                nc.tensor.transpose(tp[hr:hr + 64, :], onrm[:], ident[:])
                nc.vector.tensor_copy(
                    concat[(ch, hp)][hr:hr + 64, bass.ts(j, 128)],
                    tp[hr:hr + 64, :])

            def emit_wo_b(ch):
                """Second half (heads 2-3) + combine + staging DMAs + RS."""
                dst = rs_in[ch][:].rearrange(
                    "(s mp p) t -> p mp s t", s=4, p=128)
                for m in range(NKC):
                    fo = fbpool.tile([128, QC], BF16, tag="fob", name="fob")
                    if ch == NCH - 1 and m % 2:
                        # tail only: the score PSUM pool is idle by now;
                        # alternating pools doubles the Wo pipeline depth
                        ps = ps_sc.tile([128, 2 * QC], F32, tag="sc",
                                        name="wobps2")[:, 0:QC]
                    else:
                        ps = ps_mm.tile([128, QC], F32, tag="mm",
                                        name="wobps")
                    for kc in range(2):
                        nc.tensor.matmul(
                            ps[:], wo_sb[:, kc * D + m * 128:kc * D + (m + 1) * 128],
                            concat[(ch, kc)][:],
                            start=(kc == 0), stop=(kc == 1),
                        )
                    if ch == NCH - 1 and m % 2:
                        nc.scalar.activation(
                            fo[:], ps[:],
                            mybir.ActivationFunctionType.Identity,
                            bias=bo4_sb[:, m:m + 1])
                    else:
                        nc.vector.tensor_scalar_add(fo[:], ps[:],
                                                    bo4_sb[:, m:m + 1])
                    nc.sync.dma_start(
                        dst[:, m, :, :],
                        fo[:].rearrange("p (s t) -> p s t", s=4))
                nc.gpsimd.collective_compute(
                    "ReduceScatter", mybir.AluOpType.add,
                    replica_groups=[[0, 1, 2, 3], [4, 5, 6, 7]],
                    ins=[rs_in[ch][:].opt()],
                    outs=[rs_out[ch][:].opt()],
                )
                # collectives may not write IO tensors; bounce via DRAM on
                # the gpsimd queue (its head is already gated on this RS)
                nc.gpsimd.dma_start(outs[ch][:], rs_out[ch][:])

            # extras[(sub, slot)] -> emissions filling early PE gaps while
            # respecting DMA arrival order
            extras = {
                (0, 0): [lambda: qk_proj(wk_sb, 1, bk_sb, kh_sb, 1)],
                (0, 2): [lambda: qk_proj(wk_sb, 1, bk_sb, kh_sb, 2)],
                (0, 4): [lambda: qk_proj(wk_sb, 1, bk_sb, kh_sb, 3)],
                (1, 0): [lambda: qk_proj(wq_sb, 0, bq_sb, qh_sb, 1)],
                (6, 0): [lambda: qk_proj(wq_sb, 0, bq_sb, qh_sb, 2)],
                (10, 0): [lambda: qk_proj(wq_sb, 0, bq_sb, qh_sb, 3)],
            }
            for i in range(16):   # V-proj 128-token groups over subs 2-3
                extras.setdefault((2 + i // 8, i % 8), []).append(
                    (lambda q, jj: lambda: v_proj_j(q, jj))(i // 4, i % 4))

            # sub-block order: chunk-major, heads inner.  Consumers: none
            # for subs 0-3 (projection catch-up), two per sub for 4-6, one
            # per sub afterwards.
            subs = [(ch, h) for ch in range(NCH) for h in range(HPC)]
            pending = []      # emitted-scores sub-blocks awaiting consumers
            done_h = {ch: 0 for ch in range(NCH)}

            def consume_one():
                st = pending.pop(0)
                for j in range(QC // 128):
                    emit_consumer_step(st, j)
                done_h[st.ch] += 1
                if done_h[st.ch] == HPC:
                    emit_wo_b(st.ch)

            # consume schedule: none during projection catch-up (subs 0-3),
            # one per sub mid-stream (depth ~4), two per sub near the end
            # (subs 12-14) so only sub 15's consumer trails the exp stream
            for si, (ch, h) in enumerate(subs):
                sc2 = []
                n_consume = 0 if si < 4 else (2 if 12 <= si < 15 else 1)
                for ktp in range(NKT // 2):
                    sc2.append(emit_scores(ch, h, ktp))
                    for fn in extras.get((si, ktp), ()):
                        fn()
                    if n_consume and ktp in ((1, 5) if n_consume == 2 else (5,)):
                        consume_one()
                pending.append(Sub(ch, h, sc2))
            while pending:
                consume_one()

    nc.compile()
    _CACHE["nc"] = nc
    return nc


def _prep_inputs(q, k, v, Wq, bq, Wk, bk, Wv, bv, Wo, bo):
    """Build the per-core input maps (host-side sharding)."""
    in_maps = []
    for c in range(N_CORES):
        b, hg = c // 4, c % 4
        fsl = slice(FEAT * hg, FEAT * (hg + 1))
        wv_aug = np.zeros((D, HPC * VW), np.float32)
        bv_aug = np.zeros((HPC * VW,), np.float32)
        for h in range(HPC):
            rows = slice(FEAT * hg + DK * h, FEAT * hg + DK * (h + 1))
            wv_aug[:, h * VW:h * VW + DK] = Wv[rows, :].T
            bv_aug[h * VW:h * VW + DK] = bv[rows]
            bv_aug[h * VW + DK] = 1.0
        bqko = np.concatenate([
            bq[fsl].reshape(2, 128).T,
            bk[fsl].reshape(2, 128).T,
            (bo * 0.25).reshape(8, 128).T,
        ], axis=1)
        in_maps.append({
            "xq": np.ascontiguousarray(q[b].T).astype(NPBF16),
            "xk": np.ascontiguousarray(k[b].T).astype(NPBF16),
            "xv": np.ascontiguousarray(v[b].T).astype(NPBF16),
            "wq": np.ascontiguousarray(Wq[fsl].T).astype(NPBF16),
            "wk": np.ascontiguousarray(Wk[fsl].T).astype(NPBF16),
            "wv": wv_aug.astype(NPBF16),
            "wo": np.ascontiguousarray(Wo[:, fsl].T).astype(NPBF16),
            "bqko": np.ascontiguousarray(bqko).astype(np.float32),
            "bv": bv_aug.reshape(1, HPC * VW).astype(NPBF16),
        })
    return in_maps


def run_sharded(in_maps, trace=False):
    nc = _build_program()
    res = run_bass_kernel_spmd(nc, in_maps, list(range(N_CORES)), trace=trace)
    full = np.empty((B, S, D), np.float32)
    for c in range(N_CORES):
        b, hg = c // 4, c % 4
        for ch in range(NCH):
            t0 = QC * ch + RSTOK * hg
            full[b, t0:t0 + RSTOK, :] = \
                res.results[c][f"out{ch}"].astype(np.float32).T
    return full, res


def kernel(q, k, v, Wq, bq, Wk, bk, Wv, bv, Wo, bo):
    args = [np.asarray(x, np.float32) for x in
            (q, k, v, Wq, bq, Wk, bk, Wv, bv, Wo, bo)]
    in_maps = _prep_inputs(*args)
    full, _ = run_sharded(in_maps)
    return full


# revision 4
# speedup vs baseline: 1.0353x; 1.0073x over previous
"""Multi-head attention (B=2, S=2048, D=1024, H=16) on 8 Trainium2 cores, v3.

Sharding: core c handles batch c//4 and head-group c%4 (4 heads x dk 64).
Pipeline:
  1. x loaded in 512-token quarter DMAs (one DMA per tensor-quarter),
     weights in one DMA per tensor.  Only the projections the first score
     tiles need run up front; K quarters 1-3, V, and Q quarters 1-3 are
     interleaved into early attention sub-blocks.
  2. Q/K projections -> [128(2 heads), 2048] feature-major layout, bias
     fused into the PSUM->SBUF DVE copy.  V projection -> token-major
     [128 tok, 4x(64+1)] with a ones column per head (softmax denominator).
  3. Attention in 16 sub-blocks (512-query chunk x head), software
     pipelined with variable depth: the exp stream runs several sub-blocks
     ahead early (while the PE drains projection work), consumers catch up
     to depth 1 mid-stream.  Scores for a key-tile pair share one
     [128, 1024] PSUM pair -> one exp per pair.
  4. Wo partial projection (256 local features -> full D) + bo/4 bias per
     512-token chunk, staged to DRAM, one ReduceScatter per chunk across
     the 4-core batch group (4 staggered collectives; only the last is
     exposed).  RS writes bf16 output params directly; host upcasts.
"""

import numpy as np
import ml_dtypes

import concourse.bass as bass
import concourse.tile as tile
from concourse import bacc, mybir
from concourse.bass_utils import run_bass_kernel_spmd

BF16 = mybir.dt.bfloat16
F32 = mybir.dt.float32
NPBF16 = ml_dtypes.bfloat16

B, S, D, H = 2, 2048, 1024, 16
DK = 64
N_CORES = 8
HPC = 4                 # heads per core
FEAT = HPC * DK         # 256 projected features per core
VW = DK + 1             # 65: per-head v columns incl ones
TOKQ = 512              # token quarter for x DMAs / projections
QC = 512                # query chunk per attention sub-block
NCH = S // QC           # 4 query chunks
NKC = D // 128          # 8 contraction chunks
NKT = S // 128          # 16 key tiles
RSTOK = QC // 4         # 128 tokens per core per ReduceScatter

_CACHE = {}


def _build_program():
    if "nc" in _CACHE:
        return _CACHE["nc"]

    from concourse.masks import make_identity

    nc = bacc.Bacc("TRN2", target_bir_lowering=False, debug=False,
                   num_devices=N_CORES)

    xq = nc.declare_dram_parameter("xq", [D, S], BF16, isOutput=False)
    xk = nc.declare_dram_parameter("xk", [D, S], BF16, isOutput=False)
    xv = nc.declare_dram_parameter("xv", [D, S], BF16, isOutput=False)
    wq = nc.declare_dram_parameter("wq", [D, FEAT], BF16, isOutput=False)
    wk = nc.declare_dram_parameter("wk", [D, FEAT], BF16, isOutput=False)
    wv = nc.declare_dram_parameter("wv", [D, HPC * VW], BF16, isOutput=False)
    wo = nc.declare_dram_parameter("wo", [FEAT, D], BF16, isOutput=False)
    # merged per-partition bias columns: bq(2) | bk(2) | bo/4(8)
    bqko = nc.declare_dram_parameter("bqko", [128, 12], F32, isOutput=False)
    bv = nc.declare_dram_parameter("bv", [1, HPC * VW], BF16, isOutput=False)
    # ReduceScatter writes each 128-token output piece directly; host
    # upcasts bf16 -> f32
    outs = [nc.declare_dram_parameter(f"out{r}", [D, RSTOK], BF16,
                                      isOutput=True) for r in range(NCH)]

    with tile.TileContext(nc) as tc:
        with (
            tc.tile_pool(name="w", bufs=1) as wpool,
            tc.tile_pool(name="x", bufs=2) as xpool,
            tc.tile_pool(name="qk", bufs=1) as qkpool,
            tc.tile_pool(name="sc", bufs=42) as scpool,
            tc.tile_pool(name="sm", bufs=4) as smpool,
            tc.tile_pool(name="cat", bufs=1) as catpool,
            tc.tile_pool(name="fb", bufs=9) as fbpool,
            tc.tile_pool(name="ps_sc", bufs=2, space="PSUM") as ps_sc,
            tc.tile_pool(name="ps_pv", bufs=2, space="PSUM") as ps_pv,
            tc.tile_pool(name="ps_mm", bufs=2, space="PSUM") as ps_mm,
            tc.tile_pool(name="dram", bufs=1, space="DRAM") as dram,
        ):
            # ---- constants / weights (one DMA per tensor) ----------------
            ident = wpool.tile([128, 128], BF16, tag="ident")
            make_identity(nc, ident[:])
            ones1 = wpool.tile([1, 128], BF16, tag="ones")
            nc.vector.memset(ones1[:], 1.0)

            def kc_load(name, dramt, width):
                t = wpool.tile([128, NKC * width], BF16, tag=name, name=name)
                nc.sync.dma_start(
                    t[:].rearrange("p (kc f) -> p kc f", f=width),
                    dramt[:].rearrange("(kc p) f -> p kc f", p=128))
                return t

            x_tiles = {}  # (tensor_idx, quarter) -> tile

            def x_dma(ti, xdram, qtr):
                src = xdram[:].rearrange("(kc p) t -> p kc t", p=128)
                t = xpool.tile([128, NKC * TOKQ], BF16, tag=f"x{ti}",
                               name=f"x{ti}_{qtr}")
                nc.sync.dma_start(
                    t[:].rearrange("p (kc t) -> p kc t", t=TOKQ),
                    src[:, :, bass.ts(qtr, TOKQ)])
                x_tiles[(ti, qtr)] = t

            # DMA stream ordered so the first score matmuls start early;
            # later K/V/Q quarters arrive while attention runs.
            wk_sb = kc_load("wks", wk, FEAT)
            x_dma(1, xk, 0)
            wq_sb = kc_load("wqs", wq, FEAT)
            bqko_sb = wpool.tile([128, 12], F32, tag="bqko")
            nc.sync.dma_start(bqko_sb[:], bqko[:])
            bq_sb = bqko_sb[:, 0:2]
            bk_sb = bqko_sb[:, 2:4]
            bo4_sb = bqko_sb[:, 4:12]
            x_dma(0, xq, 0)
            for qtr in range(1, 4):
                x_dma(1, xk, qtr)
            x_dma(0, xq, 1)
            wv_sb = kc_load("wvs", wv, HPC * VW)
            bv_sb = wpool.tile([1, HPC * VW], BF16, tag="bv")
            nc.sync.dma_start(bv_sb[:], bv[:])
            for qtr in range(4):
                x_dma(2, xv, qtr)
            wo_sb = wpool.tile([128, 2 * D], BF16, tag="wos")
            nc.sync.dma_start(
                wo_sb[:].rearrange("p (kc f) -> p kc f", f=D),
                wo[:].rearrange("(kc p) f -> p kc f", p=128))
            for qtr in range(2, 4):
                x_dma(0, xq, qtr)

            # ---- projections --------------------------------------------
            qh_sb = [qkpool.tile([128, S], BF16, tag=f"qh{m}", name=f"qh{m}")
                     for m in range(2)]
            kh_sb = [qkpool.tile([128, S], BF16, tag=f"kh{m}", name=f"kh{m}")
                     for m in range(2)]
            v_big = qkpool.tile([128, NKT * HPC * VW], BF16, tag="vb")

            def qk_proj(w_sb, ti, b_sb, dst, qtr):
                xt = x_tiles[(ti, qtr)]
                for m in range(2):
                    ps = ps_mm.tile([128, TOKQ], F32, tag="mm", name="qkps")
                    for kc in range(NKC):
                        nc.tensor.matmul(
                            ps[:],
                            w_sb[:, kc * FEAT + m * 128:kc * FEAT + (m + 1) * 128],
                            xt[:, bass.ts(kc, TOKQ)],
                            start=(kc == 0), stop=(kc == NKC - 1),
                        )
                    nc.vector.tensor_scalar_add(
                        dst[m][:, bass.ts(qtr, TOKQ)], ps[:], b_sb[:, m:m + 1])

            # V bias materialized once across all 128 partitions so the
            # per-group bias add rides the PSUM->SBUF copy as tensor_tensor;
            # emitted lazily (bv arrives mid-stream)
            bv_full = qkpool.tile([128, HPC * VW], BF16, tag="bvf")

            def bv_prep():
                ps0 = ps_mm.tile([128, TOKQ], F32, tag="mm", name="bvps")
                nc.tensor.matmul(ps0[:, 0:HPC * VW], ones1[:], bv_sb[:],
                                 start=True, stop=True)
                nc.vector.tensor_copy(bv_full[:], ps0[:, 0:HPC * VW])

            def v_proj_j(qtr, j):
                """One 128-token group of the V projection."""
                xt = x_tiles[(2, qtr)]
                ps = ps_mm.tile([128, TOKQ], F32, tag="mm", name="vps")
                for kc in range(NKC):
                    nc.tensor.matmul(
                        ps[:, 0:HPC * VW],
                        xt[:, kc * TOKQ + j * 128:kc * TOKQ + (j + 1) * 128],
                        wv_sb[:, kc * HPC * VW:(kc + 1) * HPC * VW],
                        start=(kc == 0), stop=(kc == NKC - 1),
                    )
                kt = qtr * 4 + j
                nc.vector.tensor_tensor(
                    v_big[:, kt * HPC * VW:(kt + 1) * HPC * VW],
                    ps[:, 0:HPC * VW], bv_full[:], mybir.AluOpType.add)

            # up-front: only what sub-block 0 needs
            qk_proj(wk_sb, 1, bk_sb, kh_sb, 0)
            qk_proj(wq_sb, 0, bq_sb, qh_sb, 0)

            # ---- attention + output projection, software-pipelined ------
            rs_in = [dram.tile([4 * D, RSTOK], BF16, tag=f"rsi{r}",
                               name=f"rsi{r}") for r in range(NCH)]
            rs_out = [dram.tile([D, RSTOK], BF16, tag=f"rso{r}",
                                name=f"rso{r}") for r in range(NCH)]

            concat = {}
            for ch in range(NCH):
                for kc in range(2):
                    concat[(ch, kc)] = catpool.tile(
                        [128, QC], BF16, tag=f"cat{ch}{kc}",
                        name=f"cat{ch}{kc}")

            def emit_scores(ch, h, ktp):
                """Scores for key tiles (2*ktp, 2*ktp+1) over this 512-query
                chunk; one exp for the pair."""
                hp, hr = h // 2, (h % 2) * 64
                ps = ps_sc.tile([128, 2 * QC], F32, tag="sc", name="scps")
                for half in range(2):
                    kt = 2 * ktp + half
                    nc.tensor.matmul(
                        ps[:, bass.ts(half, QC)],
                        kh_sb[hp][hr:hr + 64, bass.ts(kt, 128)],
                        qh_sb[hp][hr:hr + 64, bass.ts(ch, QC)],
                        start=True, stop=True,
                    )
                sc = scpool.tile([128, 2 * QC], BF16, tag="sc", name="sc")
                nc.scalar.activation(
                    sc[:], ps[:], mybir.ActivationFunctionType.Exp,
                    scale=0.125)
                return sc

            class Sub:
                def __init__(self, ch, h, sc2):
                    self.ch, self.h, self.sc2 = ch, h, sc2
                    self.pv = None

            def emit_consumer_step(st, j):
                """PV accumulation for 128-query block j of a finished
                sub-block, then normalize + transpose + concat store."""
                ch, h = st.ch, st.h
                hp, hr = h // 2, (h % 2) * 64
                if st.pv is None:
                    st.pv = ps_pv.tile([128, QC], F32, tag="pv", name="pv")
                pv = st.pv
                for kt in range(NKT):
                    nc.tensor.matmul(
                        pv[:, j * 128:j * 128 + VW],
                        st.sc2[kt // 2][:, (kt % 2) * QC + j * 128:
                                        (kt % 2) * QC + (j + 1) * 128],
                        v_big[:, kt * HPC * VW + h * VW:
                              kt * HPC * VW + h * VW + VW],
                        start=(kt == 0), stop=(kt == NKT - 1),
                    )
                r = smpool.tile([128, 1], F32, tag="r8", name="r8")
                nc.vector.reciprocal(r[:], pv[:, j * 128 + DK:j * 128 + DK + 1])
                onrm = smpool.tile([128, DK], BF16, tag="onrm", name="onrm")
                nc.vector.tensor_scalar_mul(
                    onrm[:], pv[:, j * 128:j * 128 + DK], r[:])
                tp = ps_mm.tile([128, 128], BF16, tag="mm", name="tp")
                nc.tensor.transpose(tp[hr:hr + 64, :], onrm[:], ident[:])
                nc.vector.tensor_copy(
                    concat[(ch, hp)][hr:hr + 64, bass.ts(j, 128)],
                    tp[hr:hr + 64, :])

            def emit_wo_b(ch):
                """Second half (heads 2-3) + combine + staging DMAs + RS."""
                dst = rs_in[ch][:].rearrange(
                    "(s mp p) t -> p mp s t", s=4, p=128)
                for m in range(NKC):
                    fo = fbpool.tile([128, QC], BF16, tag="fob", name="fob")
                    if ch == NCH - 1 and m % 2:
                        # tail only: the score PSUM pool is idle by now;
                        # alternating pools doubles the Wo pipeline depth
                        ps = ps_sc.tile([128, 2 * QC], F32, tag="sc",
                                        name="wobps2")[:, 0:QC]
                    else:
                        ps = ps_mm.tile([128, QC], F32, tag="mm",
                                        name="wobps")
                    for kc in range(2):
                        nc.tensor.matmul(
                            ps[:], wo_sb[:, kc * D + m * 128:kc * D + (m + 1) * 128],
                            concat[(ch, kc)][:],
                            start=(kc == 0), stop=(kc == 1),
                        )
                    if ch == NCH - 1 and m % 2:
                        nc.scalar.activation(
                            fo[:], ps[:],
                            mybir.ActivationFunctionType.Identity,
                            bias=bo4_sb[:, m:m + 1])
                    else:
                        nc.vector.tensor_scalar_add(fo[:], ps[:],
                                                    bo4_sb[:, m:m + 1])
                    nc.sync.dma_start(
                        dst[:, m, :, :],
                        fo[:].rearrange("p (s t) -> p s t", s=4))
                nc.gpsimd.collective_compute(
                    "ReduceScatter", mybir.AluOpType.add,
                    replica_groups=[[0, 1, 2, 3], [4, 5, 6, 7]],
                    ins=[rs_in[ch][:].opt()],
                    outs=[rs_out[ch][:].opt()],
                )
                # collectives may not write IO tensors; bounce via DRAM on
                # the gpsimd queue (its head is already gated on this RS)
                nc.gpsimd.dma_start(outs[ch][:], rs_out[ch][:])

            # extras[(sub, slot)] -> emissions filling early PE gaps while
            # respecting DMA arrival order
            extras = {
                (0, 0): [lambda: qk_proj(wk_sb, 1, bk_sb, kh_sb, 1)],
                (0, 2): [lambda: qk_proj(wk_sb, 1, bk_sb, kh_sb, 2)],
                (0, 4): [lambda: qk_proj(wk_sb, 1, bk_sb, kh_sb, 3)],
                (1, 0): [lambda: qk_proj(wq_sb, 0, bq_sb, qh_sb, 1)],
                (1, 7): [bv_prep],
                (6, 0): [lambda: qk_proj(wq_sb, 0, bq_sb, qh_sb, 2)],
                (10, 0): [lambda: qk_proj(wq_sb, 0, bq_sb, qh_sb, 3)],
            }
            for i in range(16):   # V-proj 128-token groups over subs 2-3
                extras.setdefault((2 + i // 8, i % 8), []).append(
                    (lambda q, jj: lambda: v_proj_j(q, jj))(i // 4, i % 4))

            # sub-block order: chunk-major, heads inner.  Consumers: none
            # for subs 0-3 (projection catch-up), two per sub for 4-6, one
            # per sub afterwards.
            subs = [(ch, h) for ch in range(NCH) for h in range(HPC)]
            pending = []      # emitted-scores sub-blocks awaiting consumers
            done_h = {ch: 0 for ch in range(NCH)}

            def consume_one():
                st = pending.pop(0)
                for j in range(QC // 128):
                    emit_consumer_step(st, j)
                done_h[st.ch] += 1
                if done_h[st.ch] == HPC:
                    emit_wo_b(st.ch)

            # consume schedule: none during projection catch-up (subs 0-3),
            # one per sub mid-stream (depth ~4), two per sub near the end
            # (subs 12-14) so only sub 15's consumer trails the exp stream
            for si, (ch, h) in enumerate(subs):
                sc2 = []
                n_consume = 0 if si < 4 else (2 if 12 <= si < 15 else 1)
                for ktp in range(NKT // 2):
                    sc2.append(emit_scores(ch, h, ktp))
                    for fn in extras.get((si, ktp), ()):
                        fn()
                    if n_consume and ktp in ((1, 5) if n_consume == 2 else (5,)):
                        consume_one()
                pending.append(Sub(ch, h, sc2))
            while pending:
                consume_one()

    nc.compile()
    _CACHE["nc"] = nc
    return nc


def _prep_inputs(q, k, v, Wq, bq, Wk, bk, Wv, bv, Wo, bo):
    """Build the per-core input maps (host-side sharding)."""
    in_maps = []
    for c in range(N_CORES):
        b, hg = c // 4, c % 4
        fsl = slice(FEAT * hg, FEAT * (hg + 1))
        wv_aug = np.zeros((D, HPC * VW), np.float32)
        bv_aug = np.zeros((HPC * VW,), np.float32)
        for h in range(HPC):
            rows = slice(FEAT * hg + DK * h, FEAT * hg + DK * (h + 1))
            wv_aug[:, h * VW:h * VW + DK] = Wv[rows, :].T
            bv_aug[h * VW:h * VW + DK] = bv[rows]
            bv_aug[h * VW + DK] = 1.0
        bqko = np.concatenate([
            bq[fsl].reshape(2, 128).T,
            bk[fsl].reshape(2, 128).T,
            (bo * 0.25).reshape(8, 128).T,
        ], axis=1)
        in_maps.append({
            "xq": np.ascontiguousarray(q[b].T).astype(NPBF16),
            "xk": np.ascontiguousarray(k[b].T).astype(NPBF16),
            "xv": np.ascontiguousarray(v[b].T).astype(NPBF16),
            "wq": np.ascontiguousarray(Wq[fsl].T).astype(NPBF16),
            "wk": np.ascontiguousarray(Wk[fsl].T).astype(NPBF16),
            "wv": wv_aug.astype(NPBF16),
            "wo": np.ascontiguousarray(Wo[:, fsl].T).astype(NPBF16),
            "bqko": np.ascontiguousarray(bqko).astype(np.float32),
            "bv": bv_aug.reshape(1, HPC * VW).astype(NPBF16),
        })
    return in_maps


def run_sharded(in_maps, trace=False):
    nc = _build_program()
    res = run_bass_kernel_spmd(nc, in_maps, list(range(N_CORES)), trace=trace)
    full = np.empty((B, S, D), np.float32)
    for c in range(N_CORES):
        b, hg = c // 4, c % 4
        for ch in range(NCH):
            t0 = QC * ch + RSTOK * hg
            full[b, t0:t0 + RSTOK, :] = \
                res.results[c][f"out{ch}"].astype(np.float32).T
    return full, res


def kernel(q, k, v, Wq, bq, Wk, bk, Wv, bv, Wo, bo):
    args = [np.asarray(x, np.float32) for x in
            (q, k, v, Wq, bq, Wk, bk, Wv, bv, Wo, bo)]
    in_maps = _prep_inputs(*args)
    full, _ = run_sharded(in_maps)
    return full
